# revision 47
# baseline (speedup 1.0000x reference)
"""Dinov3 ViT attention (B=4, N=1024, D=1024, H=16, HD=64) on 8 TRN2
NeuronCores, written against the Bass/Tile stack.

Sharding: core c -> (batch b = c//2, head-group g = c%2, 8 heads each).
Each core computes q/k/v projections for its 512-feature slice, rotary,
attention, and a partial o_proj (its head-group's wo columns). The host
sums the two partials per batch and adds the constant bias vector
(bo + bv @ wo.T — exact, since softmax rows sum to 1).

Device-side schedule (per core, fp16 matmuls / fp32 accumulation):
  phase 0   V projections (token-major, into 128-wide per-head slots of
            v65 whose column 0 is ones -> the AV matmul emits softmax
            denominators in psum row 0 for free), with pair-0 Q/K
            projections threaded in.
  pairs     per head: scores S^T = K_rot^T-major @ Q_rot (feature-major
            layouts straight from the projections, so no transposes
            anywhere), exp on ACT straight out of psum (constant -3 shift
            keeps fp16 exp in range; softmax is shift-invariant), AV
            accumulation; the NEXT pair's Q/K projections + rotary run
            inside this pair's ACT-bound stretch to keep the PE warm.
  o_proj    token-major partial output, DMA'd per 128-row block.

Host passes pre-transposed/sliced fp16 inputs (transposes are free on
host): xt = hidden[b].T, D-major weight slices, cos/sin patterns with the
rotate_half sign baked in. PSUM budget: pq 2 + pss 4 + pav 2 = 8 banks.
"""

import sys

if "/opt/trn_rl_repo" not in sys.path:
    sys.path.insert(0, "/opt/trn_rl_repo")

import numpy as np

import concourse.bass as bass
import concourse.bacc as bacc
import concourse.mybir as mybir
from concourse import tile
from concourse import bass_utils
from contextlib import ExitStack

B, N, D = 4, 1024, 1024
H, HD = 16, 64
F = 512          # per-core feature slice (8 heads)
P = 128
NKB = 8          # contraction blocks over D
NTB = 8          # token blocks of 128
NH = 8           # local heads
MODE = "f16"     # "f16" | "bf16" | "f32r"

_CACHE = {}


def build_nc(mode="f16", debug=False):
    assert mode in ("f16", "bf16", "f32r")
    if mode == "f16":
        dt = mybir.dt.float16
    elif mode == "bf16":
        dt = mybir.dt.bfloat16
    else:
        dt = mybir.dt.float32r
    f32 = mybir.dt.float32
    AF = mybir.ActivationFunctionType
    ALU = mybir.AluOpType

    nc = bacc.Bacc("TRN2", target_bir_lowering=False, debug=False, num_devices=8)
    xt_d = nc.dram_tensor("xt", (P, NKB * N), dt, kind="ExternalInput").ap()
    wqt_d = nc.dram_tensor("wqt", (P, NKB * F), dt, kind="ExternalInput").ap()
    wkt_d = nc.dram_tensor("wkt", (P, NKB * F), dt, kind="ExternalInput").ap()
    wvt_d = nc.dram_tensor("wvt", (P, NKB * F), dt, kind="ExternalInput").ap()
    wot_d = nc.dram_tensor("wot", (P, 4 * D), dt, kind="ExternalInput").ap()
    bq_d = nc.dram_tensor("bq", (P, 4), f32, kind="ExternalInput").ap()
    cs_d = nc.dram_tensor("cs", (P, N), dt, kind="ExternalInput").ap()
    ss_d = nc.dram_tensor("ss", (P, N), dt, kind="ExternalInput").ap()
    out_d = nc.dram_tensor("out", (N, D), f32, kind="ExternalOutput").ap()

    with tile.TileContext(nc) as tc, ExitStack() as top:
        pool = top.enter_context(tc.tile_pool(name="sb", bufs=1))

        cs_sb = pool.tile([P, N], dt, name="cs")
        ss_sb = pool.tile([P, N], dt, name="ss")
        bq_sb = pool.tile([P, 4], f32, name="bq")
        ebias = pool.tile([P, 1], f32, name="ebias")
        xt_big = pool.tile([P, NKB * N], dt, name="xtb")
        wq_big = pool.tile([P, NKB * F], dt, name="wqb")
        wk_big = pool.tile([P, NKB * F], dt, name="wkb")
        wv_big = pool.tile([P, NKB * F], dt, name="wvb")
        wot_big = pool.tile([P, 4 * D], dt, name="wotb")
        xt_sb = [xt_big[:, k * N:(k + 1) * N] for k in range(NKB)]
        wq_sb = [wq_big[:, k * F:(k + 1) * F] for k in range(NKB)]
        wk_sb = [wk_big[:, k * F:(k + 1) * F] for k in range(NKB)]
        wv_sb = [wv_big[:, k * F:(k + 1) * F] for k in range(NKB)]
        wot_sb = [wot_big[:, m * D:(m + 1) * D] for m in range(4)]
        qt_sb = [pool.tile([P, N], dt, name=f"qt{m}") for m in range(4)]
        kt_sb = [pool.tile([P, N], dt, name=f"kt{m}") for m in range(4)]
        v65_sb = [pool.tile([P, NH * 128], dt, name=f"v65_{t}") for t in range(NTB)]
        ot_sb = [pool.tile([P, N], dt, name=f"ot{m}") for m in range(4)]
        rcp_h = [pool.tile([1, N], f32, name=f"rcp{h}") for h in range(NH)]

        nc.any.memset(ebias[:], -3.0 if mode == "f16" else 0.0)

        nc.sync.dma_start(bq_sb[:], bq_d)
        nc.sync.dma_start(wv_big[:], wvt_d)
        nc.scalar.dma_start(xt_big[:], xt_d)
        nc.scalar.dma_start(wq_big[:], wqt_d)
        nc.sync.dma_start(wk_big[:], wkt_d)
        nc.sync.dma_start(cs_sb[:], cs_d)
        nc.sync.dma_start(ss_sb[:], ss_d)

        swp = top.enter_context(tc.tile_pool(name="swp", bufs=2))
        ptp = top.enter_context(tc.tile_pool(name="ptp", bufs=18))
        rbp = top.enter_context(tc.tile_pool(name="rbp", bufs=2))
        ost = top.enter_context(tc.tile_pool(name="ost", bufs=3))

        pq = top.enter_context(tc.tile_pool(name="pq", bufs=1, space="PSUM"))

        def qk_proj(m, which):
            """Full Q or K projection for pair m: 16 MMs, halves alternating."""
            w_sb, dst_sb = (wq_sb, qt_sb) if which == "q" else (wk_sb, kt_sb)
            ps = pq.tile([P, N], f32, tag="pq", name="psqk")
            for k in range(NKB):
                for half in range(2):
                    hs = slice(half * F, (half + 1) * F)
                    nc.tensor.matmul(
                        ps[:, hs], w_sb[k][:, m * P:(m + 1) * P],
                        xt_sb[k][:, hs],
                        start=(k == 0), stop=(k == NKB - 1))
            if which == "q":
                nc.vector.tensor_scalar_add(dst_sb[m][:], ps[:],
                                            bq_sb[:, m:m + 1])
            else:
                nc.vector.tensor_copy(dst_sb[m][:], ps[:])

        def rotary(m, src_sb):
            sw = swp.tile([P, N], dt, tag="sw", name="sw")
            for blk in range(4):
                o = blk * 32
                nc.gpsimd.dma_start(sw[o:o + 32, :],
                                    src_sb[m][o ^ 32:(o ^ 32) + 32, :])
            nc.vector.tensor_tensor(sw[:], sw[:], ss_sb[:], op=ALU.mult)
            nc.vector.tensor_tensor(src_sb[m][:], src_sb[m][:], cs_sb[:],
                                    op=ALU.mult)
            nc.vector.tensor_tensor(src_sb[m][:], src_sb[m][:], sw[:],
                                    op=ALU.add)

        pss = top.enter_context(tc.tile_pool(name="pss", bufs=2, space="PSUM"))

        def s_step(m, kb, pts):
            ps2 = [pss.tile([P, N], f32, tag="pss", name="pss")
                   for _ in range(2)]
            for qh in range(2):
                qs = slice(qh * F, (qh + 1) * F)
                for par in range(2):
                    off = par * 64
                    nc.tensor.matmul(
                        ps2[par][:, qs],
                        kt_sb[m][off:off + 64, kb * P:(kb + 1) * P],
                        qt_sb[m][off:off + 64, qs],
                        start=True, stop=True, tile_position=(off, 0))
            for par in range(2):
                ptile = ptp.tile([P, N], dt, tag="pt", name="ptile", bufs=18)
                nc.scalar.activation(ptile[:], ps2[par][:], AF.Exp,
                                     scale=0.125, bias=ebias[:])
                pts[par].append(ptile)

        # ---------- phase 0: V + QK(0) + early S(0) steps ----------
        pts0 = ([], [])
        with ExitStack() as ph0:
            pv = ph0.enter_context(tc.tile_pool(name="pv", bufs=2, space="PSUM"))
            for t in range(NTB):
                ps = pv.tile([P, F], f32, tag="pv", name="psv")
                for k in range(NKB):
                    nc.tensor.matmul(
                        ps[:], xt_sb[k][:, t * P:(t + 1) * P], wv_sb[k][:],
                        start=(k == 0), stop=(k == NKB - 1))
                nc.any.memset(v65_sb[t][:], 1.0)
                dst = v65_sb[t][:].rearrange("p (h e) -> p h e", e=128)[:, :, 64:128]
                nc.vector.tensor_copy(dst,
                                      ps[:].rearrange("p (h e) -> p h e", e=64))
                if t == 2:
                    qk_proj(0, "q")
                elif t == 3:
                    qk_proj(0, "k")
                    rotary(0, qt_sb)
                    rotary(0, kt_sb)
                elif t >= 4:
                    s_step(0, t - 4, pts0)
        nc.sync.dma_start(wot_big[:], wot_d)

        # ---------- attention with interleaved next-pair projections ----------
        with ExitStack() as phc:
            pav = phc.enter_context(tc.tile_pool(name="pav", bufs=1, space="PSUM"))
            for m in range(4):
                pts = pts0 if m == 0 else ([], [])
                for kb in range(4 if m == 0 else 0, NTB):
                    s_step(m, kb, pts)
                for par in range(2):
                    h = 2 * m + par
                    off = par * 64
                    av = pav.tile([P, N], f32, tag="pav", name="av")
                    for kb in range(NTB):
                        for qh in range(2):
                            qs = slice(qh * F, (qh + 1) * F)
                            nc.tensor.matmul(
                                av[:, qs],
                                v65_sb[kb][:, h * 128:(h + 1) * 128],
                                pts[par][kb][:, qs],
                                start=(kb == 0), stop=(kb == NTB - 1))
                    nc.scalar.copy(ot_sb[m][off:off + 64, :], av[64:128, :])
                    nc.vector.reciprocal_approx_fast(rcp_h[h][:], av[0:1, :])
                    # thread next pair's projections into ACT-bound stretch
                    if m < 3:
                        if par == 0:
                            qk_proj(m + 1, "q")
                            rotary(m + 1, qt_sb)
                        else:
                            qk_proj(m + 1, "k")
                            rotary(m + 1, kt_sb)

                rbe = rbp.tile([P, N], f32, tag="rb", name="rbe")
                rbo = rbp.tile([P, N], f32, tag="rb", name="rbo")
                nc.gpsimd.partition_broadcast(rbe[:], rcp_h[2 * m][:])
                nc.gpsimd.partition_broadcast(rbo[:], rcp_h[2 * m + 1][:])
                nc.vector.tensor_tensor(ot_sb[m][0:64, :], ot_sb[m][0:64, :],
                                        rbe[0:64, :], op=ALU.mult)
                nc.vector.tensor_tensor(ot_sb[m][64:128, :],
                                        ot_sb[m][64:128, :],
                                        rbo[64:128, :], op=ALU.mult)

        # ---------- o_proj ----------
        if True:
            for qb in range(NTB):
                ps = pss.tile([P, D], f32, tag="pss", name="pso")
                for mm_ in range(4):
                    for half in range(2):
                        hs = slice(half * F, (half + 1) * F)
                        nc.tensor.matmul(
                            ps[:, hs],
                            ot_sb[mm_][:, qb * P:(qb + 1) * P],
                            wot_sb[mm_][:, hs],
                            start=(mm_ == 0), stop=(mm_ == 3))
                ot_out = ost.tile([P, D], f32, tag="oout", name="oout")
                nc.scalar.copy(ot_out[:], ps[:])
                eng = nc.sync if qb % 2 == 0 else nc.scalar
                eng.dma_start(out_d[qb * P:(qb + 1) * P, :], ot_out[:])

    nc.compile()
    return nc


def host_prep(inputs, mode=MODE):
    """Slice/transpose full inputs into 8 per-core input maps."""
    hs = np.asarray(inputs["hidden_states"], np.float32)
    cos = np.asarray(inputs["cos"], np.float32)
    sin = np.asarray(inputs["sin"], np.float32)
    wq = np.asarray(inputs["wq"], np.float32)
    wk = np.asarray(inputs["wk"], np.float32)
    wv = np.asarray(inputs["wv"], np.float32)
    wo = np.asarray(inputs["wo"], np.float32)
    bq = np.asarray(inputs["bq"], np.float32)

    if mode == "bf16":
        import ml_dtypes
        cast = lambda a: np.ascontiguousarray(a).astype(ml_dtypes.bfloat16)
    elif mode == "f16":
        cast = lambda a: np.ascontiguousarray(a).astype(np.float16)
    else:
        cast = lambda a: np.ascontiguousarray(a, np.float32)

    sgn = np.ones((64, 1), np.float32)
    sgn[:32] = -1.0
    in_maps = []
    for c in range(8):
        b, g = c // 2, c % 2
        fs = slice(g * F, (g + 1) * F)
        csx = cos[b].T  # (64, N)
        ssx = sin[b].T * sgn
        pack = lambda a, kd: np.ascontiguousarray(
            a.reshape(kd, P, -1).transpose(1, 0, 2).reshape(P, -1))
        in_maps.append({
            "xt": cast(pack(hs[b].T, NKB)),
            "wqt": cast(pack(wq[fs, :].T, NKB)),
            "wkt": cast(pack(wk[fs, :].T, NKB)),
            "wvt": cast(pack(wv[fs, :].T, NKB)),
            "wot": cast(pack(wo[:, fs].T, 4)),
            "bq": np.ascontiguousarray(bq[fs].reshape(4, P).T, np.float32),
            "cs": cast(np.concatenate([csx, csx], axis=0)),
            "ss": cast(np.concatenate([ssx, ssx], axis=0)),
        })
    return in_maps


def host_finish(results, inputs):
    bo = np.asarray(inputs["bo"], np.float32)
    bv = np.asarray(inputs["bv"], np.float32)
    wo = np.asarray(inputs["wo"], np.float32)
    const = bo + bv @ wo.T
    out = np.empty((B, N, D), np.float32)
    for b in range(B):
        out[b] = results[2 * b]["out"] + results[2 * b + 1]["out"] + const
    return out


def _get_nc(mode=MODE):
    if mode not in _CACHE:
        _CACHE[mode] = build_nc(mode)
    return _CACHE[mode]


def run(inputs, mode=MODE, trace=False, tmpdir=None):
    nc = _get_nc(mode)
    in_maps = host_prep(inputs, mode)
    res = bass_utils.run_bass_kernel_spmd(
        nc, in_maps, core_ids=list(range(8)), trace=trace, tmpdir=tmpdir)
    return host_finish(res.results, inputs), res


def kernel(**inputs):
    out, _ = run(inputs)
    return out


# revision 48
# speedup vs baseline: 1.0349x; 1.0349x over previous
"""Dinov3 ViT attention (B=4, N=1024, D=1024, H=16, HD=64) on 8 TRN2
NeuronCores, written against the Bass/Tile stack.

Sharding: core c -> (batch b = c//2, head-group g = c%2, 8 heads each).
Each core computes q/k/v projections for its 512-feature slice, rotary,
attention, and a partial o_proj (its head-group's wo columns). The host
sums the two partials per batch and adds the constant bias vector
(bo + bv @ wo.T — exact, since softmax rows sum to 1).

Device-side schedule (per core, fp16 matmuls / fp32 accumulation):
  phase 0   V projections (token-major, into 128-wide per-head slots of
            v65 whose column 0 is ones -> the AV matmul emits softmax
            denominators in psum row 0 for free), with pair-0 Q/K
            projections threaded in.
  pairs     per head: scores S^T = K_rot^T-major @ Q_rot (feature-major
            layouts straight from the projections, so no transposes
            anywhere), exp on ACT straight out of psum (constant -3 shift
            keeps fp16 exp in range; softmax is shift-invariant), AV
            accumulation; the NEXT pair's Q/K projections + rotary run
            inside this pair's ACT-bound stretch to keep the PE warm.
  o_proj    token-major partial output, DMA'd per 128-row block.

Host passes pre-transposed/sliced fp16 inputs (transposes are free on
host): xt = hidden[b].T, D-major weight slices, cos/sin patterns with the
rotate_half sign baked in. PSUM budget: pq 2 + pss 4 + pav 2 = 8 banks.
"""

import sys

if "/opt/trn_rl_repo" not in sys.path:
    sys.path.insert(0, "/opt/trn_rl_repo")

import numpy as np

import concourse.bass as bass
import concourse.bacc as bacc
import concourse.mybir as mybir
from concourse import tile
from concourse import bass_utils
from contextlib import ExitStack

B, N, D = 4, 1024, 1024
H, HD = 16, 64
F = 512          # per-core feature slice (8 heads)
P = 128
NKB = 8          # contraction blocks over D
NTB = 8          # token blocks of 128
NH = 8           # local heads
MODE = "f16"     # "f16" | "bf16" | "f32r"

_CACHE = {}


def build_nc(mode="f16", debug=False):
    assert mode in ("f16", "bf16", "f32r")
    if mode == "f16":
        dt = mybir.dt.float16
    elif mode == "bf16":
        dt = mybir.dt.bfloat16
    else:
        dt = mybir.dt.float32r
    f32 = mybir.dt.float32
    AF = mybir.ActivationFunctionType
    ALU = mybir.AluOpType

    nc = bacc.Bacc("TRN2", target_bir_lowering=False, debug=False, num_devices=8)
    xt_d = nc.dram_tensor("xt", (P, NKB * N), dt, kind="ExternalInput").ap()
    wqt_d = nc.dram_tensor("wqt", (P, NKB * F), dt, kind="ExternalInput").ap()
    wkt_d = nc.dram_tensor("wkt", (P, NKB * F), dt, kind="ExternalInput").ap()
    wvt_d = nc.dram_tensor("wvt", (P, NKB * F), dt, kind="ExternalInput").ap()
    wot_d = nc.dram_tensor("wot", (P, 4 * D), dt, kind="ExternalInput").ap()
    bq_d = nc.dram_tensor("bq", (P, 4), f32, kind="ExternalInput").ap()
    cs_d = nc.dram_tensor("cs", (P, N), dt, kind="ExternalInput").ap()
    ss_d = nc.dram_tensor("ss", (P, N), dt, kind="ExternalInput").ap()
    out_d = nc.dram_tensor("out", (N, D), f32, kind="ExternalOutput").ap()

    with tile.TileContext(nc) as tc, ExitStack() as top:
        pool = top.enter_context(tc.tile_pool(name="sb", bufs=1))

        cs_sb = pool.tile([P, N], dt, name="cs")
        ss_sb = pool.tile([P, N], dt, name="ss")
        bq_sb = pool.tile([P, 4], f32, name="bq")
        ebias = pool.tile([P, 1], f32, name="ebias")
        xt_big = pool.tile([P, NKB * N], dt, name="xtb")
        wq_big = pool.tile([P, NKB * F], dt, name="wqb")
        wk_big = pool.tile([P, NKB * F], dt, name="wkb")
        wv_big = pool.tile([P, NKB * F], dt, name="wvb")
        wot_big = pool.tile([P, 4 * D], dt, name="wotb")
        xt_sb = [xt_big[:, k * N:(k + 1) * N] for k in range(NKB)]
        wq_sb = [wq_big[:, k * F:(k + 1) * F] for k in range(NKB)]
        wk_sb = [wk_big[:, k * F:(k + 1) * F] for k in range(NKB)]
        wv_sb = [wv_big[:, k * F:(k + 1) * F] for k in range(NKB)]
        wot_sb = [wot_big[:, m * D:(m + 1) * D] for m in range(4)]
        qt_sb = [pool.tile([P, N], dt, name=f"qt{m}") for m in range(4)]
        kt_sb = [pool.tile([P, N], dt, name=f"kt{m}") for m in range(4)]
        v65_sb = [pool.tile([P, NH * 128], dt, name=f"v65_{t}") for t in range(NTB)]
        ot_sb = [pool.tile([P, N], dt, name=f"ot{m}") for m in range(4)]
        rcp_h = [pool.tile([1, N], f32, name=f"rcp{h}") for h in range(NH)]

        nc.any.memset(ebias[:], -3.0 if mode == "f16" else 0.0)

        nc.sync.dma_start(bq_sb[:], bq_d)
        nc.sync.dma_start(wv_big[:], wvt_d)
        nc.scalar.dma_start(xt_big[:], xt_d)
        nc.scalar.dma_start(wq_big[:], wqt_d)
        nc.sync.dma_start(wk_big[:], wkt_d)
        nc.sync.dma_start(cs_sb[:], cs_d)
        nc.sync.dma_start(ss_sb[:], ss_d)

        swp = top.enter_context(tc.tile_pool(name="swp", bufs=2))
        ptp = top.enter_context(tc.tile_pool(name="ptp", bufs=18))
        rbp = top.enter_context(tc.tile_pool(name="rbp", bufs=2))
        ost = top.enter_context(tc.tile_pool(name="ost", bufs=3))

        pq = top.enter_context(tc.tile_pool(name="pq", bufs=1, space="PSUM"))

        def qk_proj(m, which):
            """Full Q or K projection for pair m: 16 MMs, halves alternating."""
            w_sb, dst_sb = (wq_sb, qt_sb) if which == "q" else (wk_sb, kt_sb)
            ps = pq.tile([P, N], f32, tag="pq", name="psqk")
            for k in range(NKB):
                for half in range(2):
                    hs = slice(half * F, (half + 1) * F)
                    nc.tensor.matmul(
                        ps[:, hs], w_sb[k][:, m * P:(m + 1) * P],
                        xt_sb[k][:, hs],
                        start=(k == 0), stop=(k == NKB - 1))
            if which == "q":
                nc.vector.tensor_scalar_add(dst_sb[m][:], ps[:],
                                            bq_sb[:, m:m + 1])
            else:
                nc.vector.tensor_copy(dst_sb[m][:], ps[:])

        def rotary(m, src_sb):
            sw = swp.tile([P, N], dt, tag="sw", name="sw")
            for blk in range(4):
                o = blk * 32
                nc.gpsimd.dma_start(sw[o:o + 32, :],
                                    src_sb[m][o ^ 32:(o ^ 32) + 32, :])
            nc.vector.tensor_tensor(sw[:], sw[:], ss_sb[:], op=ALU.mult)
            nc.vector.tensor_tensor(src_sb[m][:], src_sb[m][:], cs_sb[:],
                                    op=ALU.mult)
            nc.vector.tensor_tensor(src_sb[m][:], src_sb[m][:], sw[:],
                                    op=ALU.add)

        # ---------- phase 0: V projections, QK(0) threaded in early ----------
        with ExitStack() as ph0:
            pv = ph0.enter_context(tc.tile_pool(name="pv", bufs=2, space="PSUM"))
            for t in range(NTB):
                ps = pv.tile([P, F], f32, tag="pv", name="psv")
                for k in range(NKB):
                    nc.tensor.matmul(
                        ps[:], xt_sb[k][:, t * P:(t + 1) * P], wv_sb[k][:],
                        start=(k == 0), stop=(k == NKB - 1))
                nc.any.memset(v65_sb[t][:], 1.0)
                dst = v65_sb[t][:].rearrange("p (h e) -> p h e", e=128)[:, :, 64:128]
                nc.vector.tensor_copy(dst,
                                      ps[:].rearrange("p (h e) -> p h e", e=64))
                if t == 2:
                    qk_proj(0, "q")
                elif t == 3:
                    qk_proj(0, "k")
                    rotary(0, qt_sb)
                    rotary(0, kt_sb)
        nc.sync.dma_start(wot_big[:], wot_d)

        # ---------- attention with interleaved next-pair projections ----------
        with ExitStack() as phc:
            pss = phc.enter_context(tc.tile_pool(name="pss", bufs=2, space="PSUM"))
            pav = phc.enter_context(tc.tile_pool(name="pav", bufs=1, space="PSUM"))
            for m in range(4):
                pts = ([], [])
                for kb in range(NTB):
                    ps2 = [pss.tile([P, N], f32, tag="pss", name="pss")
                           for _ in range(2)]
                    for qh in range(2):
                        qs = slice(qh * F, (qh + 1) * F)
                        for par in range(2):
                            off = par * 64
                            nc.tensor.matmul(
                                ps2[par][:, qs],
                                kt_sb[m][off:off + 64, kb * P:(kb + 1) * P],
                                qt_sb[m][off:off + 64, qs],
                                start=True, stop=True,
                                tile_position=(off, 0))
                    for par in range(2):
                        ptile = ptp.tile([P, N], dt, tag="pt", name="ptile",
                                         bufs=18)
                        nc.scalar.activation(ptile[:], ps2[par][:], AF.Exp,
                                             scale=0.125, bias=ebias[:])
                        pts[par].append(ptile)
                for par in range(2):
                    h = 2 * m + par
                    off = par * 64
                    av = pav.tile([P, N], f32, tag="pav", name="av")
                    for kb in range(NTB):
                        for qh in range(2):
                            qs = slice(qh * F, (qh + 1) * F)
                            nc.tensor.matmul(
                                av[:, qs],
                                v65_sb[kb][:, h * 128:(h + 1) * 128],
                                pts[par][kb][:, qs],
                                start=(kb == 0), stop=(kb == NTB - 1))
                    nc.scalar.copy(ot_sb[m][off:off + 64, :], av[64:128, :])
                    nc.vector.reciprocal_approx_fast(rcp_h[h][:], av[0:1, :])
                    # thread next pair's projections into ACT-bound stretch
                    if m < 3:
                        if par == 0:
                            qk_proj(m + 1, "q")
                            rotary(m + 1, qt_sb)
                        else:
                            qk_proj(m + 1, "k")
                            rotary(m + 1, kt_sb)

                rbe = rbp.tile([P, N], f32, tag="rb", name="rbe")
                rbo = rbp.tile([P, N], f32, tag="rb", name="rbo")
                nc.gpsimd.partition_broadcast(rbe[:], rcp_h[2 * m][:])
                nc.gpsimd.partition_broadcast(rbo[:], rcp_h[2 * m + 1][:])
                nc.vector.tensor_tensor(ot_sb[m][0:64, :], ot_sb[m][0:64, :],
                                        rbe[0:64, :], op=ALU.mult)
                nc.vector.tensor_tensor(ot_sb[m][64:128, :],
                                        ot_sb[m][64:128, :],
                                        rbo[64:128, :], op=ALU.mult)

        # ---------- o_proj ----------
        with ExitStack() as phe:
            po = phe.enter_context(tc.tile_pool(name="po", bufs=2, space="PSUM"))
            for qb in range(NTB):
                ps = po.tile([P, D], f32, tag="po", name="pso")
                for mm_ in range(4):
                    for half in range(2):
                        hs = slice(half * F, (half + 1) * F)
                        nc.tensor.matmul(
                            ps[:, hs],
                            ot_sb[mm_][:, qb * P:(qb + 1) * P],
                            wot_sb[mm_][:, hs],
                            start=(mm_ == 0), stop=(mm_ == 3))
                ot_out = ost.tile([P, D], f32, tag="oout", name="oout")
                nc.scalar.copy(ot_out[:], ps[:])
                eng = nc.sync if qb % 2 == 0 else nc.scalar
                eng.dma_start(out_d[qb * P:(qb + 1) * P, :], ot_out[:])

    nc.compile()
    return nc


def host_prep(inputs, mode=MODE):
    """Slice/transpose full inputs into 8 per-core input maps."""
    hs = np.asarray(inputs["hidden_states"], np.float32)
    cos = np.asarray(inputs["cos"], np.float32)
    sin = np.asarray(inputs["sin"], np.float32)
    wq = np.asarray(inputs["wq"], np.float32)
    wk = np.asarray(inputs["wk"], np.float32)
    wv = np.asarray(inputs["wv"], np.float32)
    wo = np.asarray(inputs["wo"], np.float32)
    bq = np.asarray(inputs["bq"], np.float32)

    if mode == "bf16":
        import ml_dtypes
        cast = lambda a: np.ascontiguousarray(a).astype(ml_dtypes.bfloat16)
    elif mode == "f16":
        cast = lambda a: np.ascontiguousarray(a).astype(np.float16)
    else:
        cast = lambda a: np.ascontiguousarray(a, np.float32)

    sgn = np.ones((64, 1), np.float32)
    sgn[:32] = -1.0
    in_maps = []
    for c in range(8):
        b, g = c // 2, c % 2
        fs = slice(g * F, (g + 1) * F)
        csx = cos[b].T  # (64, N)
        ssx = sin[b].T * sgn
        pack = lambda a, kd: np.ascontiguousarray(
            a.reshape(kd, P, -1).transpose(1, 0, 2).reshape(P, -1))
        in_maps.append({
            "xt": cast(pack(hs[b].T, NKB)),
            "wqt": cast(pack(wq[fs, :].T, NKB)),
            "wkt": cast(pack(wk[fs, :].T, NKB)),
            "wvt": cast(pack(wv[fs, :].T, NKB)),
            "wot": cast(pack(wo[:, fs].T, 4)),
            "bq": np.ascontiguousarray(bq[fs].reshape(4, P).T, np.float32),
            "cs": cast(np.concatenate([csx, csx], axis=0)),
            "ss": cast(np.concatenate([ssx, ssx], axis=0)),
        })
    return in_maps


def host_finish(results, inputs):
    bo = np.asarray(inputs["bo"], np.float32)
    bv = np.asarray(inputs["bv"], np.float32)
    wo = np.asarray(inputs["wo"], np.float32)
    const = bo + bv @ wo.T
    out = np.empty((B, N, D), np.float32)
    for b in range(B):
        out[b] = results[2 * b]["out"] + results[2 * b + 1]["out"] + const
    return out


def _get_nc(mode=MODE):
    if mode not in _CACHE:
        _CACHE[mode] = build_nc(mode)
    return _CACHE[mode]


def run(inputs, mode=MODE, trace=False, tmpdir=None):
    nc = _get_nc(mode)
    in_maps = host_prep(inputs, mode)
    res = bass_utils.run_bass_kernel_spmd(
        nc, in_maps, core_ids=list(range(8)), trace=trace, tmpdir=tmpdir)
    return host_finish(res.results, inputs), res


def kernel(**inputs):
    out, _ = run(inputs)
    return out
